# revision 2
# baseline (speedup 1.0000x reference)
"""FCOS head postprocessing on 8 Trainium2 NeuronCores.

Data-parallel over the batch: each of the 8 cores decodes 2 of the 16 images
(sigmoid/max/score over all 17064 anchor points, the memory-bound bulk of the
work) with a Bass kernel.  Top-k selection and NMS for each image are applied
to the per-point scores on the host.

Self-contained: hardcodes all shapes from the problem spec.
"""
import numpy as np

B = 16
NUM_CLASSES = 80
LEVEL_HW = [(100, 128), (50, 64), (25, 32), (13, 16), (7, 8)]
STRIDES = [8, 16, 32, 64, 128]
N = sum(h * w for h, w in LEVEL_HW)  # 17064
N_CORES = 8
IMGS_PER_CORE = B // N_CORES  # 2
SCORE_THR = 0.5
NMS_THR = 0.5
MAX_BOX = 1000

_COMPILED = {}


def _build_kernel():
    """Bass kernel: for each of 2 images, compute per-point
    max-class-logit (f32), sigmoid(max), sigmoid(cen), score.
    Input layout per image: cls [80, N] (levels concatenated), cen [1, N].
    Output: score [2, N] and max-logit argmax helper data [2, N] (max value).
    """
    import concourse.bass as bass
    import concourse.mybir as mybir
    import concourse.tile as tile
    from concourse import bacc

    AF = mybir.ActivationFunctionType
    ALU = mybir.AluOpType

    nc = bacc.Bacc("TRN2", debug=False, num_devices=N_CORES)

    cls_in = nc.dram_tensor("cls", [IMGS_PER_CORE, NUM_CLASSES, N],
                            mybir.dt.float32, kind="ExternalInput")
    cen_in = nc.dram_tensor("cen", [IMGS_PER_CORE, N],
                            mybir.dt.float32, kind="ExternalInput")
    score_out = nc.dram_tensor("score", [IMGS_PER_CORE, N],
                               mybir.dt.float32, kind="ExternalOutput")

    P = 128
    F = 134  # ceil(N/128) columns in wrapped layout
    NPAD = P * F  # 17152

    with tile.TileContext(nc) as tc:
        with tc.tile_pool(name="cls", bufs=2) as cls_pool, \
             tc.tile_pool(name="psum", bufs=4, space="PSUM") as psum_pool, \
             tc.tile_pool(name="wrk", bufs=2) as wrk, \
             tc.tile_pool(name="cst", bufs=1) as cst:

            ident = cst.tile([P, P], mybir.dt.float32, tag="ident")
            nc.vector.memset(ident[:], 1.0)
            # keep in_ (1.0) where iota(p - f) == 0, else fill 0 -> identity
            nc.gpsimd.affine_select(
                ident[:], ident[:], pattern=[[-1, P]],
                compare_op=ALU.is_equal, fill=0.0,
                base=0, channel_multiplier=1)

            for img in range(IMGS_PER_CORE):
                # ---- load cls [81 rows, NPAD]: rows 0-79 cls, row 80 cen
                ct = cls_pool.tile([81, NPAD], mybir.dt.float32, tag="ct")
                nc.vector.memset(ct[:, N:], -30.0)
                nc.sync.dma_start(ct[0:NUM_CLASSES, 0:N], cls_in[img])
                nc.sync.dma_start(ct[80:81, 0:N], cen_in[img:img + 1])

                # ---- transpose chunks [81,128] -> PSUM [128,81], reduce max
                M = wrk.tile([P, F], mybir.dt.float32, tag="M")    # max logit
                CW = wrk.tile([P, F], mybir.dt.float32, tag="CW")  # cen wrapped
                CH_PER_TILE = 5
                n_tiles = (F + CH_PER_TILE - 1) // CH_PER_TILE  # 27
                for t in range(n_tiles):
                    c0 = t * CH_PER_TILE
                    cw = min(CH_PER_TILE, F - c0)
                    pt = psum_pool.tile([P, CH_PER_TILE * 81], mybir.dt.float32,
                                        tag="pt")
                    for k in range(cw):
                        nc.tensor.transpose(
                            pt[:, (k * 81):(k * 81 + 81)],
                            ct[:, (c0 + k) * P:(c0 + k + 1) * P],
                            ident[0:81, 0:81])
                    # segmented max over the 80 class cols of each chunk
                    src = pt[:].rearrange("p (c a) -> p c a", a=81)
                    nc.vector.tensor_reduce(
                        M[:, c0:c0 + cw], src[:, 0:cw, 0:80],
                        axis=mybir.AxisListType.X, op=ALU.max)
                    # cen column (index 80 of each chunk)
                    nc.vector.tensor_copy(CW[:, c0:c0 + cw], src[:, 0:cw, 80])

                # ---- sigmoid(M), sigmoid(CW) — bitwise-matches jax-on-neuron:
                # sig(x) = reciprocal(1 + exp(-x))
                SM = wrk.tile([P, F], mybir.dt.float32, tag="SM")
                SC = wrk.tile([P, F], mybir.dt.float32, tag="SC")
                Q = wrk.tile([P, F], mybir.dt.float32, tag="Q")
                nc.scalar.activation(SM[:], M[:], AF.Exp, scale=-1.0)
                nc.vector.tensor_scalar_add(SM[:], SM[:], 1.0)
                nc.vector.reciprocal(SM[:], SM[:])
                nc.scalar.activation(SC[:], CW[:], AF.Exp, scale=-1.0)
                nc.vector.tensor_scalar_add(SC[:], SC[:], 1.0)
                nc.vector.reciprocal(SC[:], SC[:])
                nc.vector.tensor_tensor(Q[:], SM[:], SC[:], ALU.mult)
                nc.scalar.activation(Q[:], Q[:], AF.Sqrt)

                # ---- write back: score wrapped [128,134] -> linear [N]
                # element (p, c) is point i = 128*c + p; N = 133*128 + 40
                so = score_out[img, 0:(133 * P)].rearrange("(c p) -> c p", p=P)
                nc.sync.dma_start(so.rearrange("c p -> p c"), Q[:, 0:133])
                nc.sync.dma_start(score_out[img, (133 * P):N], Q[0:40, 133])

    nc.compile()
    return nc


def _get_nc():
    if "nc" not in _COMPILED:
        _COMPILED["nc"] = _build_kernel()
    return _COMPILED["nc"]


def _coords():
    out = []
    for (h, w), s in zip(LEVEL_HW, STRIDES):
        xs = np.arange(w, dtype=np.float32) * s
        ys = np.arange(h, dtype=np.float32) * s
        yy, xx = np.meshgrid(ys, xs, indexing="ij")
        out.append(np.stack([xx.ravel(), yy.ravel()], -1) + s // 2)
    return np.concatenate(out, 0).astype(np.float32)


def kernel(**inputs):
    from concourse.bass_utils import run_bass_kernel_spmd

    cls = np.concatenate(
        [inputs[f"cls{i}"].reshape(B, NUM_CLASSES, -1) for i in range(5)], axis=2)
    cen = np.concatenate(
        [inputs[f"cen{i}"].reshape(B, 1, -1) for i in range(5)], axis=2)[:, 0]
    reg = np.concatenate(
        [inputs[f"reg{i}"].reshape(B, 4, -1) for i in range(5)], axis=2)

    nc = _get_nc()
    in_maps = []
    for c in range(N_CORES):
        sl = slice(c * IMGS_PER_CORE, (c + 1) * IMGS_PER_CORE)
        in_maps.append({
            "cls": np.ascontiguousarray(cls[sl]),
            "cen": np.ascontiguousarray(cen[sl]),
        })
    res = run_bass_kernel_spmd(nc, in_maps, core_ids=list(range(N_CORES)))

    score = np.concatenate([r["score"] for r in res.results], axis=0)  # [B, N]

    # ---- host: argmax (cls logits), top-k, NMS (numpy, f32-exact like jax)
    coords = _coords()
    classes_all = cls.argmax(axis=1).astype(np.int32) + 1  # [B, N]

    k = MAX_BOX
    scores_out = np.zeros((B, k), np.float32)
    classes_out = np.zeros((B, k), np.int32)
    boxes_out = np.zeros((B, k, 4), np.float32)

    for b in range(B):
        s = score[b]
        # top-k sorted desc, ties by index asc (matches lax.top_k)
        ti = np.lexsort((np.arange(N), -s.astype(np.float64)))[:k]
        top_s = s[ti]
        top_c = classes_all[b][ti]
        x1 = coords[ti, 0] - reg[b, 0, ti]
        y1 = coords[ti, 1] - reg[b, 1, ti]
        x2 = coords[ti, 0] + reg[b, 2, ti]
        y2 = coords[ti, 1] + reg[b, 3, ti]
        boxes = np.stack([x1, y1, x2, y2], -1).astype(np.float32)

        max_c = np.float32(np.abs(boxes).max() + np.float32(1.0))
        bb = (boxes + top_c[:, None].astype(np.float32) * max_c).astype(np.float32)
        X1, Y1, X2, Y2 = bb[:, 0], bb[:, 1], bb[:, 2], bb[:, 3]
        area = (X2 - X1) * (Y2 - Y1)
        iw = np.maximum(np.minimum(X2[:, None], X2[None, :])
                        - np.maximum(X1[:, None], X1[None, :]), np.float32(0.0))
        ih = np.maximum(np.minimum(Y2[:, None], Y2[None, :])
                        - np.maximum(Y1[:, None], Y1[None, :]), np.float32(0.0))
        inter = iw * ih
        iou = inter / (area[:, None] + area[None, :] - inter + np.float32(1e-9))
        S = (iou > NMS_THR) & (np.arange(k)[:, None] < np.arange(k)[None, :])
        keep = top_s >= SCORE_THR
        for i in range(k):
            if keep[i]:
                keep &= ~S[i]
        scores_out[b] = np.where(keep, top_s, 0.0)
        classes_out[b] = np.where(keep, top_c, 0)
        boxes_out[b] = np.where(keep[:, None], boxes, 0.0)

    return scores_out, classes_out, boxes_out


# revision 4
# speedup vs baseline: 1.0855x; 1.0855x over previous
"""FCOS head postprocessing on 8 Trainium2 NeuronCores.

Data-parallel over the batch: each of the 8 cores decodes 2 of the 16 images
(max-over-classes via PE-transpose chunks + DVE segmented reduce, then
sigmoid/score computed so they bitwise-match jax-on-neuron) with a Bass
kernel.  Top-k selection and NMS per image run on the host on the
device-computed scores.

Self-contained: hardcodes all shapes from the problem spec.
"""
import numpy as np

B = 16
NUM_CLASSES = 80
LEVEL_HW = [(100, 128), (50, 64), (25, 32), (13, 16), (7, 8)]
STRIDES = [8, 16, 32, 64, 128]
HWs = [h * w for h, w in LEVEL_HW]
N = sum(HWs)  # 17064
N_CORES = 8
IMGS_PER_CORE = B // N_CORES  # 2
SCORE_THR = 0.5
NMS_THR = 0.5
MAX_BOX = 1000

P = 128
CHUNKS = [(hw + P - 1) // P for hw in HWs]  # 100, 25, 7, 2, 1
FCOLS = sum(CHUNKS)                         # 135
COL_OFF = [0]
for c in CHUNKS:
    COL_OFF.append(COL_OFF[-1] + c)

_COMPILED = {}


def _build_kernel():
    import concourse.mybir as mybir
    import concourse.tile as tile
    from concourse import bacc

    AF = mybir.ActivationFunctionType
    ALU = mybir.AluOpType

    nc = bacc.Bacc("TRN2", debug=False, num_devices=N_CORES)

    cls_ins = [nc.dram_tensor(f"cls{l}", [IMGS_PER_CORE, NUM_CLASSES, HWs[l]],
                              mybir.dt.float32, kind="ExternalInput")
               for l in range(5)]
    cen_ins = [nc.dram_tensor(f"cen{l}", [IMGS_PER_CORE, 1, HWs[l]],
                              mybir.dt.float32, kind="ExternalInput")
               for l in range(5)]
    # wrapped score output: element (img, p, c) = point 128*c' + p of its level
    score_out = nc.dram_tensor("score", [IMGS_PER_CORE, P, FCOLS],
                               mybir.dt.float32, kind="ExternalOutput")

    CH = 6

    with tile.TileContext(nc) as tc:
        with tc.tile_pool(name="cls", bufs=2) as cls_pool, \
             tc.tile_pool(name="psum", bufs=4, space="PSUM") as psum_pool, \
             tc.tile_pool(name="wrk", bufs=2) as wrk, \
             tc.tile_pool(name="cst", bufs=1) as cst:

            ident = cst.tile([P, P], mybir.dt.float32, tag="ident")
            nc.vector.memset(ident[:], 1.0)
            nc.gpsimd.affine_select(ident[:], ident[:], pattern=[[-1, P]],
                                    compare_op=ALU.is_equal, fill=0.0,
                                    base=0, channel_multiplier=1)

            for img in range(IMGS_PER_CORE):
                M = wrk.tile([P, FCOLS], mybir.dt.float32, tag="M")
                CW = wrk.tile([P, FCOLS], mybir.dt.float32, tag="CW")
                Q = wrk.tile([P, FCOLS], mybir.dt.float32, tag="Q")

                # per-level tiles so transposes of level l start as soon as
                # its DMA lands (overlap with later levels' loads)
                cts = []
                for l in range(5):
                    npadl = CHUNKS[l] * P
                    ctl = cls_pool.tile([81, npadl], mybir.dt.float32,
                                        tag=f"ct{l}")
                    if npadl != HWs[l]:
                        nc.vector.memset(ctl[:, HWs[l]:], -30.0)
                    nc.sync.dma_start(ctl[0:NUM_CLASSES, 0:HWs[l]],
                                      cls_ins[l][img])
                    nc.sync.dma_start(ctl[80:81, 0:HWs[l]],
                                      cen_ins[l][img, 0:1])
                    cts.append(ctl)
                chunk_src = [(cts[l], k) for l in range(5)
                             for k in range(CHUNKS[l])]

                n_tiles = (FCOLS + CH - 1) // CH
                for t in range(n_tiles):
                    c0 = t * CH
                    cw = min(CH, FCOLS - c0)
                    pt = psum_pool.tile([P, CH * 81], mybir.dt.float32,
                                        tag="pt")
                    for k in range(cw):
                        srctile, kk = chunk_src[c0 + k]
                        nc.tensor.transpose(
                            pt[:, k * 81:k * 81 + 81],
                            srctile[:, kk * P:(kk + 1) * P],
                            ident[0:81, 0:81])
                    src = pt[:].rearrange("p (c a) -> p c a", a=81)
                    nc.vector.tensor_reduce(
                        M[:, c0:c0 + cw], src[:, 0:cw, 0:80],
                        axis=mybir.AxisListType.X, op=ALU.max)
                    # cen: first sigmoid step (exp(-x)) on ACT from PSUM
                    nc.scalar.activation(CW[:, c0:c0 + cw], src[:, 0:cw, 80],
                                         AF.Exp, scale=-1.0)

                # sigmoid = reciprocal(1 + exp(-x)) — bitwise jax-on-neuron
                SM = wrk.tile([P, FCOLS], mybir.dt.float32, tag="SM")
                SC = wrk.tile([P, FCOLS], mybir.dt.float32, tag="SC")
                nc.scalar.activation(SM[:], M[:], AF.Exp, scale=-1.0)
                nc.vector.tensor_scalar_add(SM[:], SM[:], 1.0)
                nc.vector.reciprocal(SM[:], SM[:])
                nc.vector.tensor_scalar_add(SC[:], CW[:], 1.0)
                nc.vector.reciprocal(SC[:], SC[:])
                nc.vector.tensor_tensor(Q[:], SM[:], SC[:], ALU.mult)
                nc.scalar.activation(Q[:], Q[:], AF.Sqrt)

                nc.sync.dma_start(score_out[img], Q[:])

    nc.compile()
    return nc


def _get_nc():
    if "nc" not in _COMPILED:
        _COMPILED["nc"] = _build_kernel()
    return _COMPILED["nc"]


def _coords():
    out = []
    for (h, w), s in zip(LEVEL_HW, STRIDES):
        xs = np.arange(w, dtype=np.float32) * s
        ys = np.arange(h, dtype=np.float32) * s
        yy, xx = np.meshgrid(ys, xs, indexing="ij")
        out.append(np.stack([xx.ravel(), yy.ravel()], -1) + s // 2)
    return np.concatenate(out, 0).astype(np.float32)


def _unwrap(wr):
    """[P, FCOLS] wrapped -> [N] linear (per level: point = 128*c + p)."""
    parts = []
    for l in range(5):
        blk = wr[:, COL_OFF[l]:COL_OFF[l + 1]]     # [128, chunks_l]
        parts.append(blk.T.reshape(-1)[:HWs[l]])
    return np.concatenate(parts)


def kernel(**inputs):
    from concourse.bass_utils import run_bass_kernel_spmd

    nc = _get_nc()
    in_maps = []
    for c in range(N_CORES):
        sl = slice(c * IMGS_PER_CORE, (c + 1) * IMGS_PER_CORE)
        m = {}
        for l in range(5):
            m[f"cls{l}"] = np.ascontiguousarray(
                np.asarray(inputs[f"cls{l}"]).reshape(B, NUM_CLASSES, -1)[sl])
            m[f"cen{l}"] = np.ascontiguousarray(
                np.asarray(inputs[f"cen{l}"]).reshape(B, 1, -1)[sl])
        in_maps.append(m)
    res = run_bass_kernel_spmd(nc, in_maps, core_ids=list(range(N_CORES)))

    score = np.stack([_unwrap(r["score"][i])
                      for r in res.results for i in range(IMGS_PER_CORE)])  # [B, N]

    # ---- host: argmax, top-k, NMS (f32 numpy, matches reference bitwise)
    cls = np.concatenate(
        [np.asarray(inputs[f"cls{i}"]).reshape(B, NUM_CLASSES, -1)
         for i in range(5)], axis=2)
    reg = np.concatenate(
        [np.asarray(inputs[f"reg{i}"]).reshape(B, 4, -1)
         for i in range(5)], axis=2)
    coords = _coords()
    classes_all = cls.argmax(axis=1).astype(np.int32) + 1  # [B, N]

    k = MAX_BOX
    scores_out = np.zeros((B, k), np.float32)
    classes_out = np.zeros((B, k), np.int32)
    boxes_out = np.zeros((B, k, 4), np.float32)

    for b in range(B):
        s = score[b]
        ti = np.lexsort((np.arange(N), -s.astype(np.float64)))[:k]
        top_s = s[ti]
        top_c = classes_all[b][ti]
        x1 = coords[ti, 0] - reg[b, 0, ti]
        y1 = coords[ti, 1] - reg[b, 1, ti]
        x2 = coords[ti, 0] + reg[b, 2, ti]
        y2 = coords[ti, 1] + reg[b, 3, ti]
        boxes = np.stack([x1, y1, x2, y2], -1).astype(np.float32)

        max_c = np.float32(np.abs(boxes).max() + np.float32(1.0))
        bb = (boxes + top_c[:, None].astype(np.float32) * max_c).astype(np.float32)
        X1, Y1, X2, Y2 = bb[:, 0], bb[:, 1], bb[:, 2], bb[:, 3]
        area = (X2 - X1) * (Y2 - Y1)
        iw = np.maximum(np.minimum(X2[:, None], X2[None, :])
                        - np.maximum(X1[:, None], X1[None, :]), np.float32(0.0))
        ih = np.maximum(np.minimum(Y2[:, None], Y2[None, :])
                        - np.maximum(Y1[:, None], Y1[None, :]), np.float32(0.0))
        inter = iw * ih
        iou = inter / (area[:, None] + area[None, :] - inter + np.float32(1e-9))
        S = (iou > NMS_THR) & (np.arange(k)[:, None] < np.arange(k)[None, :])
        keep = top_s >= SCORE_THR
        for i in range(k):
            if keep[i]:
                keep &= ~S[i]
        scores_out[b] = np.where(keep, top_s, 0.0)
        classes_out[b] = np.where(keep, top_c, 0)
        boxes_out[b] = np.where(keep[:, None], boxes, 0.0)

    return scores_out, classes_out, boxes_out
